# revision 3
# baseline (speedup 1.0000x reference)
"""Trainium2 Bass kernel v2 for nn_DSWNV_84387517432212 (WaveNet vocoder).

Sharding: 8 cores = 4 batches x 2 time-halves; each core computes output
cols [650, 4775) of its extended range (halo recompute, per-layer trimmed).

v2 vs baseline:
  - all matmul operands bf16 (FWL weight loads, half the SBUF/DMA), PSUM f32
  - dilated-conv weights resident in SBUF for all 9 layers (loaded once)
  - full-width layer sweeps, h ping-pong in SBUF (no DRAM tail round-trips)
  - per-layer halo trimming (layer l computes only cols [OL[l], 4775))
  - causal conv emits the +1-shifted copy of h channels 128..191 directly
    (extra weight columns), removing the layer-0 shift DMA
  - skip convs restricted to the output range and software-pipelined one
    subtile behind the gated layer compute
"""

import numpy as np
import ml_dtypes

import concourse.mybir as mybir
import concourse.tile as tile
from concourse import bacc
from concourse.bass_utils import run_bass_kernel_spmd

F32 = mybir.dt.float32
BF = mybir.dt.bfloat16
AF = mybir.ActivationFunctionType
ALU = mybir.AluOpType
BF_NP = ml_dtypes.bfloat16

# model dims
B, T, NQ, HID, KK, UP, TAUX, NCOND = 4, 8249, 256, 192, 6, 110, 75, 486
DILS = [1, 6, 36, 1, 6, 36, 1, 6, 36]
# sharding / tiling
TSH = 4125
HALO = 650
W = HALO + TSH            # 4775: h cols [0, W), out cols [OUT0, W)
OUT0 = HALO
AW = W + 11               # audio cols: j = c + tap, tap<=6, +pad
SELW = W + 1              # mask col c+1 needed by causal upper bias

# per-layer first needed output column
OL = [0] * 9
OL[8] = OUT0
for _l in range(8, 0, -1):
    OL[_l - 1] = OL[_l] - 5 * DILS[_l]

# xh channel permutation: [z 0:128 | t 0:128 | z 128:192 | t 128:192]
PERM = list(range(0, 128)) + list(range(192, 320)) + \
    list(range(128, 192)) + list(range(320, 384))

P512 = [128, 512]


def _subtiles(start, end, cap=512):
    """Split [start, end) into n=ceil(W/cap) near-equal pieces (each <= cap)."""
    Wd = end - start
    n = -(-Wd // cap)
    base, rem = divmod(Wd, n)
    sizes = [base + 1] * rem + [base] * (n - rem)
    out, c = [], start
    for s in sizes:
        out.append((c, s))
        c += s
    return out


def _bf(a):
    return np.ascontiguousarray(np.asarray(a, np.float32).astype(BF_NP))


def _pack_weights(inp):
    """Host-side weight packing into SBUF-layout arrays."""
    w = {}
    cw = inp["causal_w"]                # (192, 256, 6)
    wc0 = np.zeros((128, 12, 128), np.float32)
    for tap in range(6):
        for rc in range(2):
            wc0[:, tap * 2 + rc, :] = cw[0:128, rc * 128:(rc + 1) * 128, tap].T
    # block1: out partitions 0:64 = ch128:192 @ t, 64:128 = ch128:192 @ t+1
    wc1 = np.zeros((128, 14, 128), np.float32)
    for tau in range(7):
        for rc in range(2):
            j = tau * 2 + rc
            if tau < 6:
                wc1[:, j, 0:64] = cw[128:192, rc * 128:(rc + 1) * 128, tau].T
            if tau >= 1:
                wc1[:, j, 64:128] = \
                    cw[128:192, rc * 128:(rc + 1) * 128, tau - 1].T
    w["wc0"] = _bf(wc0)
    w["wc1"] = _bf(wc1)
    cb_ = inp["causal_b"]
    bc0 = cb_[0:128].reshape(1, 128)
    bc1a = np.zeros((1, 128), np.float32)
    bc1a[0, 0:64] = cb_[128:192]
    bc1b = np.zeros((1, 128), np.float32)
    bc1b[0, 64:128] = cb_[128:192]
    w["bc0"], w["bc1a"], w["bc1b"] = _bf(bc0), _bf(bc1a), _bf(bc1b)

    dil = inp["dilh_w"][:, PERM, :, :]  # (9, 384, 192, 6)
    w["wd01"] = _bf(dil[:, :, 0:128, :].transpose(2, 0, 3, 1))  # (128,9,6,384)
    d2 = dil[:, :, 128:192, :].transpose(2, 0, 3, 1)            # (64,9,6,384)
    w["wd2p"] = _bf(np.concatenate(
        [d2[:, :, 0::2, :], d2[:, :, 1::2, :]], axis=0))        # (128,9,3,384)
    bd = inp["dilh_b"][:, PERM]
    w["bd"] = np.ascontiguousarray(
        bd.reshape(9, 3, 128).transpose(2, 0, 1)).astype(np.float32)

    sk = inp["skip_w"][:, :, :, 0]      # (9, 256, 192)
    w["ws01"] = _bf(sk[:, :, 0:128].transpose(2, 0, 1))         # (128,9,256)
    w["ws2"] = _bf(sk[:, :, 128:192].transpose(2, 0, 1))        # (64,9,256)
    w["bss"] = np.ascontiguousarray(
        inp["skip_b"].sum(0).reshape(2, 128).T).astype(np.float32)

    w["wsc"] = np.ascontiguousarray(
        inp["scale_in_w"][:, :, 0].T).astype(np.float32)
    w["bsc"] = np.asarray(inp["scale_in_b"]).reshape(54, 1).astype(np.float32)
    w["wa0"] = np.ascontiguousarray(
        inp["aux0_w"].transpose(1, 2, 0)).astype(np.float32)    # (54,3,162)
    b0 = np.zeros((128, 2), np.float32)
    b0.T.flat[:162] = inp["aux0_b"]
    w["ba0"] = b0
    a1t = inp["aux1_w"].transpose(1, 2, 0)                      # (162,3,486)
    w["wa1a"] = _bf(a1t[0:128])
    w["wa1b"] = _bf(a1t[128:162])
    b1 = np.zeros((128, 4), np.float32)
    b1.T.flat[:486] = inp["aux1_b"]
    w["ba1"] = b1

    inx = inp["inx_w"][:, :, :, 0][:, PERM, :]   # (9, 384, 486)
    wi = np.zeros((9, 4, 128, 384), np.float32)
    for r in range(4):
        n = min(128, NCOND - r * 128)
        wi[:, r, :n, :] = inx[:, :, r * 128:r * 128 + n].transpose(0, 2, 1)
    w["wi"] = _bf(wi)
    w["cb"] = _bf((inp["up_b"] * inx.sum(2) + inp["inx_b"][:, PERM])
                  .reshape(9, 1, 384))

    o1 = inp["out1_w"][:, :, 0]
    w["wo1"] = _bf(o1.T.reshape(2, 128, 256).transpose(1, 0, 2))
    w["bo1"] = np.ascontiguousarray(
        inp["out1_b"].reshape(2, 128).T).astype(np.float32)
    o2 = inp["out2_w"][:, :, 0]
    w["wo2"] = _bf(o2.T.reshape(2, 128, 256).transpose(1, 0, 2))
    w["bo2row"] = _bf(inp["out2_b"].reshape(1, 256))
    return w


def _per_core_arrays(inp, w, b, half):
    """Per-core inputs: audio shard + selector, plus shared weights."""
    t0 = 0 if half == 0 else TSH
    audio = np.zeros((NQ, AW), np.float32)
    g0 = t0 - HALO - 5                  # global t of audio col 0
    s0, s1 = max(0, g0), min(T, g0 + AW)
    audio[:, s0 - g0:s1 - g0] = inp["audio"][b, :, s0:s1]

    sel = np.zeros((76, SELW), np.float32)
    t = (t0 - HALO) + np.arange(SELW)
    valid = (t >= 0) & (t < T)
    tv = t[valid]
    sel[(tv + 1) // UP, np.where(valid)[0]] = inp["up_w"][(tv + 1) % UP]
    sel[75, valid] = 1.0

    m = {
        "audio_in": _bf(audio),
        "sel_in": _bf(sel),
        "mask_in": _bf(sel[75:76, :]),
        "aux_in": np.ascontiguousarray(inp["aux"][b]).astype(np.float32),
    }
    for k, v in w.items():
        m[k + "_in"] = np.ascontiguousarray(v)
    return m


def build_kernel():
    nc = bacc.Bacc(None, target_bir_lowering=False)
    d = {}
    shapes = {
        "audio_in": ((NQ, AW), BF), "sel_in": ((76, SELW), BF),
        "mask_in": ((1, SELW), BF),
        "aux_in": ((54, TAUX), F32),
        "wc0_in": ((128, 12, 128), BF), "wc1_in": ((128, 14, 128), BF),
        "bc0_in": ((1, 128), BF), "bc1a_in": ((1, 128), BF),
        "bc1b_in": ((1, 128), BF),
        "wd01_in": ((128, 9, 6, 384), BF), "wd2p_in": ((128, 9, 3, 384), BF),
        "bd_in": ((128, 9, 3), F32),
        "ws01_in": ((128, 9, 256), BF), "ws2_in": ((64, 9, 256), BF),
        "bss_in": ((128, 2), F32),
        "wsc_in": ((54, 54), F32), "bsc_in": ((54, 1), F32),
        "wa0_in": ((54, 3, 162), F32), "ba0_in": ((128, 2), F32),
        "wa1a_in": ((128, 3, NCOND), BF), "wa1b_in": ((34, 3, NCOND), BF),
        "ba1_in": ((128, 4), F32),
        "wi_in": ((9, 4, 128, 384), BF), "cb_in": ((9, 1, 384), BF),
        "wo1_in": ((128, 2, 256), BF), "bo1_in": ((128, 2), F32),
        "wo2_in": ((128, 2, 256), BF), "bo2row_in": ((1, 256), BF),
    }
    for k, (shp, dt) in shapes.items():
        d[k] = nc.dram_tensor(k, list(shp), dt, kind="ExternalInput")
    y_d = nc.dram_tensor("y", [TSH, NQ], F32, kind="ExternalOutput")

    def mm(out, lhsT, rhs, start, stop):
        nc.tensor.matmul(out, lhsT, rhs, start=start, stop=stop)

    with tile.TileContext(nc) as tc:
        with tc.tile_pool(name="res", bufs=1) as res:
            # ---- resident tiles + small DMAs ----
            sel_sb = res.tile([76, SELW], BF)
            nc.sync.dma_start(out=sel_sb, in_=d["sel_in"][:, :])
            mask_t = res.tile([1, SELW], BF)
            nc.sync.dma_start(out=mask_t, in_=d["mask_in"][:, :])
            mask = mask_t[:, :]
            bd_sb = res.tile([128, 9, 3], F32)
            nc.sync.dma_start(out=bd_sb, in_=d["bd_in"][:, :, :])
            ws01 = res.tile([128, 9, 256], BF)
            nc.sync.dma_start(out=ws01, in_=d["ws01_in"][:, :, :])
            ws2 = res.tile([64, 9, 256], BF)
            nc.sync.dma_start(out=ws2, in_=d["ws2_in"][:, :, :])
            bss_sb = res.tile([128, 2], F32)
            nc.sync.dma_start(out=bss_sb, in_=d["bss_in"][:, :])
            wo1 = res.tile([128, 2, 256], BF)
            nc.sync.dma_start(out=wo1, in_=d["wo1_in"][:, :, :])
            bo1_sb = res.tile([128, 2], F32)
            nc.sync.dma_start(out=bo1_sb, in_=d["bo1_in"][:, :])
            wo2 = res.tile([128, 2, 256], BF)
            nc.sync.dma_start(out=wo2, in_=d["wo2_in"][:, :, :])
            bo2row = res.tile([1, 256], BF)
            nc.sync.dma_start(out=bo2row, in_=d["bo2row_in"][:, :])

            # h ping-pong buffers (A = even-layer input, B = even-layer out)
            hA01 = res.tile([128, W], BF)
            hA2 = res.tile([128, W], BF)
            hB01 = res.tile([128, W], BF)
            hB2 = res.tile([128, W], BF)
            yaT = [res.tile([76, 384], BF, name=f"yaT{ll}") for ll in range(9)]
            wd01 = res.tile([128, 9, 6, 384], BF)
            wd2p = res.tile([128, 9, 3, 384], BF)

            # ------------- Phases A + B (scoped; audio freed after) --------
            with tc.tile_pool(name="pb", bufs=1) as pb, \
                 tc.tile_pool(name="pbs", bufs=2) as pbs, \
                 tc.tile_pool(name="ppB", bufs=1, space="PSUM") as ppB:
                # audio + causal weights first on the DMA queues
                a_t = [pb.tile([128, AW], BF, name=f"aud{i}") for i in range(2)]
                for i in range(2):
                    nc.sync.dma_start(
                        out=a_t[i][:, 0:1200],
                        in_=d["audio_in"][i * 128:(i + 1) * 128, 0:1200])
                wc0 = pb.tile([128, 12, 128], BF)
                nc.sync.dma_start(out=wc0, in_=d["wc0_in"][:, :, :])
                wc1 = pb.tile([128, 14, 128], BF)
                nc.sync.dma_start(out=wc1, in_=d["wc1_in"][:, :, :])
                bc0 = pb.tile([1, 128], BF)
                nc.sync.dma_start(out=bc0, in_=d["bc0_in"][:, :])
                bc1a = pb.tile([1, 128], BF)
                nc.sync.dma_start(out=bc1a, in_=d["bc1a_in"][:, :])
                bc1b = pb.tile([1, 128], BF)
                nc.sync.dma_start(out=bc1b, in_=d["bc1b_in"][:, :])
                for i in range(2):
                    nc.sync.dma_start(
                        out=a_t[i][:, 1200:AW],
                        in_=d["audio_in"][i * 128:(i + 1) * 128, 1200:AW])
                # big resident weight DMAs (queued behind audio)
                nc.sync.dma_start(out=wd01, in_=d["wd01_in"][:, :, :, :])
                nc.sync.dma_start(out=wd2p, in_=d["wd2p_in"][:, :, :, :])

                # ---------------- Phase A: conditioning ----------------
                with tc.tile_pool(name="ca", bufs=1) as ca, \
                     tc.tile_pool(name="cw", bufs=1) as cwp, \
                     tc.tile_pool(name="ppA", bufs=1, space="PSUM") as ppA:
                    aux_sb = ca.tile([54, TAUX], F32)
                    nc.sync.dma_start(out=aux_sb, in_=d["aux_in"][:, :])
                    wsc = ca.tile([54, 54], F32)
                    nc.sync.dma_start(out=wsc, in_=d["wsc_in"][:, :])
                    bsc = ca.tile([54, 1], F32)
                    nc.sync.dma_start(out=bsc, in_=d["bsc_in"][:, :])
                    ba0 = ca.tile([128, 2], F32)
                    nc.sync.dma_start(out=ba0, in_=d["ba0_in"][:, :])
                    ba1 = ca.tile([128, 4], F32)
                    nc.sync.dma_start(out=ba1, in_=d["ba1_in"][:, :])
                    wa0 = ca.tile([54, 3, 162], F32)
                    nc.sync.dma_start(out=wa0, in_=d["wa0_in"][:, :, :])
                    wa1a = ca.tile([128, 3, NCOND], BF)
                    nc.sync.dma_start(out=wa1a, in_=d["wa1a_in"][:, :, :])
                    wa1b = ca.tile([34, 3, NCOND], BF)
                    nc.sync.dma_start(out=wa1b, in_=d["wa1b_in"][:, :, :])

                    a0p = ppA.tile([54, TAUX], F32, tag="ap", bufs=2,
                                   padded_shape=P512)
                    mm(a0p, wsc, aux_sb, True, True)
                    a0 = ca.tile([54, TAUX], F32)
                    nc.scalar.activation(out=a0, in_=a0p, func=AF.Identity,
                                         bias=bsc)

                    a1blk = [(0, 128), (128, 34)]
                    a1 = [ca.tile([wd, TAUX], BF, name=f"a1_{i}")
                          for i, (o0, wd) in enumerate(a1blk)]
                    for i, (o0, wd) in enumerate(a1blk):
                        a1p = ppA.tile([wd, TAUX], F32, tag="ap", bufs=2,
                                       padded_shape=P512, name=f"a1p{i}")
                        ls = wa0[:, :, o0:o0 + wd]
                        mm(a1p, ls[:, 1, :], a0, True, False)
                        mm(a1p[:, 1:TAUX], ls[:, 0, :], a0[:, 0:TAUX - 1],
                           False, False)
                        mm(a1p[:, 0:TAUX - 1], ls[:, 2, :], a0[:, 1:TAUX],
                           False, True)
                        nc.scalar.activation(out=a1[i], in_=a1p,
                                             func=AF.Identity,
                                             bias=ba0[0:wd, i:i + 1])

                    a2blk = [(0, 128), (128, 128), (256, 128), (384, 102)]
                    a2 = [ca.tile([wd, TAUX], BF, name=f"a2_{i}")
                          for i, (o0, wd) in enumerate(a2blk)]
                    for i, (o0, wd) in enumerate(a2blk):
                        a2p = ppA.tile([wd, TAUX], F32, tag="ap", bufs=2,
                                       padded_shape=P512, name=f"a2p{i}")
                        for kc, wsrc in enumerate([wa1a, wa1b]):
                            ls = wsrc[:, :, o0:o0 + wd]
                            rhs = a1[kc]
                            mm(a2p, ls[:, 1, :], rhs, kc == 0, False)
                            mm(a2p[:, 3:TAUX], ls[:, 0, :], rhs[:, 0:TAUX - 3],
                               False, False)
                            mm(a2p[:, 0:TAUX - 3], ls[:, 2, :], rhs[:, 3:TAUX],
                               False, kc == 1)
                        nc.scalar.activation(out=a2[i], in_=a2p,
                                             func=AF.Identity,
                                             bias=ba1[0:wd, i:i + 1])

                    for ll in range(9):
                        wi_sb = cwp.tile([128, 4, 384], BF, tag="wi")
                        nc.sync.dma_start(
                            out=wi_sb,
                            in_=d["wi_in"][ll, :, :, :].rearrange(
                                "r p n -> p r n"))
                        yp = ppA.tile([TAUX, 384], F32, tag="yp", bufs=2,
                                      padded_shape=P512, name=f"yp{ll}")
                        for r, (o0, wd) in enumerate(a2blk):
                            mm(yp, a2[r], wi_sb[0:wd, r, :], r == 0, r == 3)
                        nc.scalar.activation(out=yaT[ll][0:TAUX, :], in_=yp,
                                             func=AF.Copy)
                        nc.sync.dma_start(out=yaT[ll][TAUX:76, :],
                                          in_=d["cb_in"][ll, :, :])

                # ------------- Phase B: causal conv + softsign -------------
                for (c0, w_) in _subtiles(0, W):
                    cc0 = ppB.tile([128, w_], F32, tag="cc", bufs=3,
                                   padded_shape=P512, name=f"cc0_{c0}")
                    for tap in range(6):
                        for rc in range(2):
                            mm(cc0, wc0[:, tap * 2 + rc, :],
                               a_t[rc][:, c0 + tap:c0 + tap + w_],
                               tap == 0 and rc == 0, False)
                    mm(cc0, bc0, mask[:, c0:c0 + w_], False, True)
                    cc1 = ppB.tile([128, w_], F32, tag="cc", bufs=3,
                                   padded_shape=P512, name=f"cc1_{c0}")
                    for tau in range(7):
                        for rc in range(2):
                            mm(cc1, wc1[:, tau * 2 + rc, :],
                               a_t[rc][:, c0 + tau:c0 + tau + w_],
                               tau == 0 and rc == 0, False)
                    mm(cc1, bc1a, mask[:, c0:c0 + w_], False, False)
                    mm(cc1, bc1b, mask[:, c0 + 1:c0 + 1 + w_], False, True)
                    for ci, (ccp, dst) in enumerate(((cc0, hA01), (cc1, hA2))):
                        ab = pbs.tile([128, w_], F32, tag="ab",
                                      padded_shape=P512)
                        nc.scalar.activation(out=ab, in_=ccp, func=AF.Abs)
                        nc.vector.tensor_scalar(out=ab, in0=ab, scalar1=1.0,
                                                scalar2=None, op0=ALU.add)
                        rr = pbs.tile([128, w_], F32, tag="rr",
                                      padded_shape=P512)
                        nc.vector.reciprocal_approx_fast(out=rr, in_=ab)
                        nc.vector.tensor_tensor(
                            out=dst[:, c0:c0 + w_], in0=ccp, in1=rr,
                            op=ALU.mult)

            # ---------------- Phases C + D ----------------
            with tc.tile_pool(name="scr", bufs=2) as scr, \
                 tc.tile_pool(name="od", bufs=1) as od, \
                 tc.tile_pool(name="pc", bufs=1, space="PSUM") as pc:
                ss = [od.tile([128, TSH], F32, name=f"ss{i}")
                      for i in range(2)]
                # layer-7 h buffers are dead once layer 8 has read them;
                # reuse as r1 storage (D trails layer 8 by >1 subtile)
                r1 = [hA01[:, 0:TSH], hA2[:, 0:TSH]]
                pend = [None]
                d_subs = _subtiles(OUT0, W)
                d_idx = [0]
                q0r = [OUT0]

                def emit_skip(ll, c0, w_, c01, c2):
                    s0 = max(c0, OUT0)
                    if s0 >= c0 + w_:
                        return
                    wv = c0 + w_ - s0
                    for ob in range(2):
                        skp = pc.tile([128, wv], F32, tag="sk", bufs=2,
                                      padded_shape=P512,
                                      name=f"skp{ll}_{c0}_{ob}")
                        mm(skp, ws01[:, ll, ob * 128:(ob + 1) * 128],
                           c01[:, s0:s0 + wv], True, False)
                        mm(skp, ws2[:, ll, ob * 128:(ob + 1) * 128],
                           c2[0:64, s0:s0 + wv], False, True)
                        sv = ss[ob][:, s0 - OUT0:s0 - OUT0 + wv]
                        if ll == 0:
                            nc.scalar.activation(out=sv, in_=skp,
                                                 func=AF.Identity,
                                                 bias=bss_sb[:, ob:ob + 1])
                        else:
                            nc.vector.tensor_tensor(out=sv, in0=skp, in1=sv,
                                                    op=ALU.add)

                def emit_o2(q0, qw):
                    o2p = pc.tile([qw, 256], F32, tag="sk", bufs=2,
                                  padded_shape=P512, name=f"o2p{q0}")
                    mm(o2p, r1[0][:, q0 - OUT0:q0 - OUT0 + qw],
                       wo2[:, 0, :], True, False)
                    mm(o2p, r1[1][:, q0 - OUT0:q0 - OUT0 + qw],
                       wo2[:, 1, :], False, False)
                    mm(o2p, mask[:, q0:q0 + qw], bo2row, False, True)
                    og = od.tile([qw, 256], F32, tag="og", bufs=2,
                                 padded_shape=[128, 256])
                    nc.scalar.activation(out=og, in_=o2p, func=AF.Copy)
                    nc.sync.dma_start(
                        out=y_d[q0 - OUT0:q0 - OUT0 + qw, :], in_=og)

                def emit_d_sub(c0, w_):
                    rl = []
                    for kc in range(2):
                        rt = scr.tile([128, w_], BF, tag=f"rl{kc}",
                                      padded_shape=P512)
                        nc.scalar.activation(
                            out=rt,
                            in_=ss[kc][:, c0 - OUT0:c0 - OUT0 + w_],
                            func=AF.Relu)
                        rl.append(rt)
                    for ob in range(2):
                        o1p = pc.tile([128, w_], F32, tag="xc", bufs=3,
                                      padded_shape=P512, name=f"o1p{c0}_{ob}")
                        mm(o1p, wo1[:, 0, ob * 128:(ob + 1) * 128],
                           rl[0], True, False)
                        mm(o1p, wo1[:, 1, ob * 128:(ob + 1) * 128],
                           rl[1], False, True)
                        nc.scalar.activation(
                            out=r1[ob][:, c0 - OUT0:c0 - OUT0 + w_],
                            in_=o1p, func=AF.Relu,
                            bias=bo1_sb[:, ob:ob + 1])

                def advance_d(x):
                    # emit D subtiles fully covered by skip-complete cols < x
                    while (d_idx[0] < len(d_subs)
                           and d_subs[d_idx[0]][0] + d_subs[d_idx[0]][1] <= x):
                        c0d, wd = d_subs[d_idx[0]]
                        emit_d_sub(c0d, wd)
                        d_idx[0] += 1
                        while q0r[0] + 128 <= c0d:
                            emit_o2(q0r[0], 128)
                            q0r[0] += 128

                def flush_pend():
                    if pend[0] is not None:
                        emit_skip(*pend[0])
                        if pend[0][0] == 8:
                            advance_d(pend[0][1] + pend[0][2])
                        pend[0] = None

                for ll in range(9):
                    dl = DILS[ll]
                    dnx = DILS[ll + 1] if ll < 8 else 0
                    if ll % 2 == 0:
                        prev01, prev2, cur01, cur2 = hA01, hA2, hB01, hB2
                    else:
                        prev01, prev2, cur01, cur2 = hB01, hB2, hA01, hA2
                    subs = _subtiles(OL[ll], W)
                    pairs = [subs[i:i + 2] for i in range(0, len(subs), 2)]
                    for pair in pairs:
                        xcsm = {}
                        for (c0, w_) in pair:
                            xcs = []
                            for mb in range(3):
                                xcp = pc.tile([128, w_], F32, tag="xc",
                                              bufs=3, padded_shape=P512,
                                              name=f"xcp{ll}_{c0}_{mb}")
                                mm(xcp, yaT[ll][:, mb * 128:(mb + 1) * 128],
                                   sel_sb[:, c0:c0 + w_], True, True)
                                xc_sb = scr.tile([128, w_], BF,
                                                 tag=f"xcs{mb}",
                                                 padded_shape=P512)
                                nc.scalar.activation(out=xc_sb, in_=xcp,
                                                     func=AF.Copy)
                                xcs.append(xc_sb)
                            xcsm[c0] = xcs
                        xhm = {c0: [] for (c0, w_) in pair}
                        for mb in range(3):
                            hcps = {}
                            for (c0, w_) in pair:
                                hcps[c0] = pc.tile(
                                    [128, w_], F32, tag="hc", bufs=3,
                                    padded_shape=P512,
                                    name=f"hcp{ll}_{c0}_{mb}")
                            for k in range(9):
                                for (c0, w_) in pair:
                                    if k < 6:
                                        off = (k - 5) * dl
                                        mm(hcps[c0],
                                           wd01[:, ll, k,
                                                mb * 128:(mb + 1) * 128],
                                           prev01[:, c0 + off:c0 + off + w_],
                                           k == 0, False)
                                    else:
                                        j = k - 6
                                        off = (2 * j - 5) * dl
                                        mm(hcps[c0],
                                           wd2p[:, ll, j,
                                                mb * 128:(mb + 1) * 128],
                                           prev2[:, c0 + off:c0 + off + w_],
                                           False, j == 2)
                            for (c0, w_) in pair:
                                xh_sb = scr.tile([128, w_], BF,
                                                 tag=f"xh{mb}",
                                                 padded_shape=P512)
                                nc.vector.scalar_tensor_tensor(
                                    out=xh_sb, in0=hcps[c0],
                                    scalar=bd_sb[:, ll, mb:mb + 1],
                                    in1=xcsm[c0][mb],
                                    op0=ALU.add, op1=ALU.mult)
                                xhm[c0].append(xh_sb)
                        for (c0, w_) in pair:
                            xh = xhm[c0]
                            flush_pend()
                            xh2b = scr.tile([64, w_], BF, tag="xh2b",
                                            padded_shape=P512)
                            nc.sync.dma_start(out=xh2b, in_=xh[2][64:128, :])
                            nc.scalar.activation(out=xh[0], in_=xh[0],
                                                 func=AF.Sigmoid)
                            nc.scalar.activation(out=xh[1], in_=xh[1],
                                                 func=AF.Tanh)
                            nc.scalar.activation(out=xh[2][0:64, :],
                                                 in_=xh[2][0:64, :],
                                                 func=AF.Sigmoid)
                            nc.scalar.activation(out=xh2b, in_=xh2b,
                                                 func=AF.Tanh)
                            dd = scr.tile([128, w_], BF, tag="dd",
                                          padded_shape=P512)
                            nc.vector.tensor_tensor(
                                out=dd, in0=prev01[:, c0:c0 + w_],
                                in1=xh[1], op=ALU.subtract)
                            nc.vector.tensor_tensor(out=dd, in0=xh[0],
                                                    in1=dd, op=ALU.mult)
                            nc.vector.tensor_tensor(
                                out=cur01[:, c0:c0 + w_],
                                in0=xh[1], in1=dd, op=ALU.add)
                            dd2 = scr.tile([64, w_], BF, tag="dd2",
                                           padded_shape=P512)
                            nc.vector.tensor_tensor(
                                out=dd2, in0=prev2[0:64, c0:c0 + w_],
                                in1=xh2b, op=ALU.subtract)
                            nc.vector.tensor_tensor(out=dd2,
                                                    in0=xh[2][0:64, :],
                                                    in1=dd2, op=ALU.mult)
                            nc.vector.tensor_tensor(
                                out=cur2[0:64, c0:c0 + w_],
                                in0=xh2b, in1=dd2, op=ALU.add)
                            if dnx:
                                nc.sync.dma_start(
                                    out=cur2[64:128, c0 - dnx:c0 - dnx + w_],
                                    in_=cur2[0:64, c0:c0 + w_])
                            pend[0] = (ll, c0, w_, cur01, cur2)
                flush_pend()
                advance_d(W + 1)
                while q0r[0] < W:
                    qw = min(128, W - q0r[0])
                    emit_o2(q0r[0], qw)
                    q0r[0] += qw
    nc.compile()
    return nc


_NC_CACHE = {}


def kernel(**inputs):
    inp = {k: np.ascontiguousarray(np.asarray(v, dtype=np.float32))
           for k, v in inputs.items()}
    if "nc" not in _NC_CACHE:
        _NC_CACHE["nc"] = build_kernel()
    nc = _NC_CACHE["nc"]
    w = _pack_weights(inp)
    in_maps = [_per_core_arrays(inp, w, core // 2, core % 2)
               for core in range(8)]
    res = run_bass_kernel_spmd(nc, in_maps, core_ids=list(range(8)))
    out = np.empty((B, T, NQ), np.float32)
    for core in range(8):
        b, half = core // 2, core % 2
        y = res.results[core]["y"]
        if half == 0:
            out[b, 0:TSH] = y
        else:
            out[b, TSH:T] = y[0:T - TSH]
    return out


# revision 4
# speedup vs baseline: 1.0310x; 1.0310x over previous
"""Trainium2 Bass kernel for nn_DSWNV_84387517432212 (WaveNet vocoder).

Sharding: 8 cores = 4 batches x 2 time-halves; each core computes output
cols [650, 4775) of its extended range (halo recompute, per-layer trimmed).

v2 vs baseline:
  - all matmul operands bf16 (FWL weight loads, half the SBUF/DMA), PSUM f32
  - dilated-conv weights resident in SBUF for all 9 layers (loaded once)
  - full-width layer sweeps, h ping-pong in SBUF (no DRAM tail round-trips)
  - per-layer halo trimming (layer l computes only cols [OL[l], 4775))
  - causal conv emits the +1-shifted copy of h channels 128..191 directly
    (extra weight columns), removing the layer-0 shift DMA
  - skip convs restricted to the output range and software-pipelined one
    subtile behind the gated layer compute
  - dilated-conv MMs grouped over subtile pairs (weight-outer order)
  - Phase D (out convs) interleaved into layer 8's sweep; out2 chunks
    interleaved behind out1 subtiles (keeps the PE clock-gate warm)
"""

import numpy as np
import ml_dtypes

import concourse.mybir as mybir
import concourse.tile as tile
from concourse import bacc
from concourse.bass_utils import run_bass_kernel_spmd

F32 = mybir.dt.float32
BF = mybir.dt.bfloat16
AF = mybir.ActivationFunctionType
ALU = mybir.AluOpType
BF_NP = ml_dtypes.bfloat16

# model dims
B, T, NQ, HID, KK, UP, TAUX, NCOND = 4, 8249, 256, 192, 6, 110, 75, 486
DILS = [1, 6, 36, 1, 6, 36, 1, 6, 36]
# sharding / tiling
TSH = 4125
HALO = 650
W = HALO + TSH            # 4775: h cols [0, W), out cols [OUT0, W)
OUT0 = HALO
AW = W + 11               # audio cols: j = c + tap, tap<=6, +pad
SELW = W + 1              # mask col c+1 needed by causal upper bias

# per-layer first needed output column
OL = [0] * 9
OL[8] = OUT0
for _l in range(8, 0, -1):
    OL[_l - 1] = OL[_l] - 5 * DILS[_l]

# xh channel permutation: [z 0:128 | t 0:128 | z 128:192 | t 128:192]
PERM = list(range(0, 128)) + list(range(192, 320)) + \
    list(range(128, 192)) + list(range(320, 384))

P512 = [128, 512]


def _subtiles(start, end, cap=512):
    """Split [start, end) into n=ceil(W/cap) near-equal pieces (each <= cap)."""
    Wd = end - start
    n = -(-Wd // cap)
    base, rem = divmod(Wd, n)
    sizes = [base + 1] * rem + [base] * (n - rem)
    out, c = [], start
    for s in sizes:
        out.append((c, s))
        c += s
    return out


def _bf(a):
    return np.ascontiguousarray(np.asarray(a, np.float32).astype(BF_NP))


def _pack_weights(inp):
    """Host-side weight packing into SBUF-layout arrays."""
    w = {}
    cw = inp["causal_w"]                # (192, 256, 6)
    wc0 = np.zeros((128, 12, 128), np.float32)
    for tap in range(6):
        for rc in range(2):
            wc0[:, tap * 2 + rc, :] = cw[0:128, rc * 128:(rc + 1) * 128, tap].T
    # block1: out partitions 0:64 = ch128:192 @ t, 64:128 = ch128:192 @ t+1
    wc1 = np.zeros((128, 14, 128), np.float32)
    for tau in range(7):
        for rc in range(2):
            j = tau * 2 + rc
            if tau < 6:
                wc1[:, j, 0:64] = cw[128:192, rc * 128:(rc + 1) * 128, tau].T
            if tau >= 1:
                wc1[:, j, 64:128] = \
                    cw[128:192, rc * 128:(rc + 1) * 128, tau - 1].T
    w["wc0"] = _bf(wc0)
    w["wc1"] = _bf(wc1)
    cb_ = inp["causal_b"]
    bc0 = cb_[0:128].reshape(1, 128)
    bc1a = np.zeros((1, 128), np.float32)
    bc1a[0, 0:64] = cb_[128:192]
    bc1b = np.zeros((1, 128), np.float32)
    bc1b[0, 64:128] = cb_[128:192]
    w["bc0"], w["bc1a"], w["bc1b"] = _bf(bc0), _bf(bc1a), _bf(bc1b)

    dil = inp["dilh_w"][:, PERM, :, :]  # (9, 384, 192, 6)
    w["wd01"] = _bf(dil[:, :, 0:128, :].transpose(2, 0, 3, 1))  # (128,9,6,384)
    d2 = dil[:, :, 128:192, :].transpose(2, 0, 3, 1)            # (64,9,6,384)
    w["wd2p"] = _bf(np.concatenate(
        [d2[:, :, 0::2, :], d2[:, :, 1::2, :]], axis=0))        # (128,9,3,384)
    bd = inp["dilh_b"][:, PERM]
    w["bd"] = np.ascontiguousarray(
        bd.reshape(9, 3, 128).transpose(2, 0, 1)).astype(np.float32)

    sk = inp["skip_w"][:, :, :, 0]      # (9, 256, 192)
    w["ws01"] = _bf(sk[:, :, 0:128].transpose(2, 0, 1))         # (128,9,256)
    w["ws2"] = _bf(sk[:, :, 128:192].transpose(2, 0, 1))        # (64,9,256)
    w["bss"] = np.ascontiguousarray(
        inp["skip_b"].sum(0).reshape(2, 128).T).astype(np.float32)

    w["wsc"] = np.ascontiguousarray(
        inp["scale_in_w"][:, :, 0].T).astype(np.float32)
    w["bsc"] = np.asarray(inp["scale_in_b"]).reshape(54, 1).astype(np.float32)
    w["wa0"] = np.ascontiguousarray(
        inp["aux0_w"].transpose(1, 2, 0)).astype(np.float32)    # (54,3,162)
    b0 = np.zeros((128, 2), np.float32)
    b0.T.flat[:162] = inp["aux0_b"]
    w["ba0"] = b0
    a1t = inp["aux1_w"].transpose(1, 2, 0)                      # (162,3,486)
    w["wa1a"] = _bf(a1t[0:128])
    w["wa1b"] = _bf(a1t[128:162])
    b1 = np.zeros((128, 4), np.float32)
    b1.T.flat[:486] = inp["aux1_b"]
    w["ba1"] = b1

    inx = inp["inx_w"][:, :, :, 0][:, PERM, :]   # (9, 384, 486)
    wi = np.zeros((9, 4, 128, 384), np.float32)
    for r in range(4):
        n = min(128, NCOND - r * 128)
        wi[:, r, :n, :] = inx[:, :, r * 128:r * 128 + n].transpose(0, 2, 1)
    w["wi"] = _bf(wi)
    w["cb"] = _bf((inp["up_b"] * inx.sum(2) + inp["inx_b"][:, PERM])
                  .reshape(9, 1, 384))

    o1 = inp["out1_w"][:, :, 0]
    w["wo1"] = _bf(o1.T.reshape(2, 128, 256).transpose(1, 0, 2))
    w["bo1"] = np.ascontiguousarray(
        inp["out1_b"].reshape(2, 128).T).astype(np.float32)
    o2 = inp["out2_w"][:, :, 0]
    w["wo2"] = _bf(o2.T.reshape(2, 128, 256).transpose(1, 0, 2))
    w["bo2row"] = _bf(inp["out2_b"].reshape(1, 256))
    return w


def _per_core_arrays(inp, w, b, half):
    """Per-core inputs: audio shard + selector, plus shared weights."""
    t0 = 0 if half == 0 else TSH
    audio = np.zeros((NQ, AW), np.float32)
    g0 = t0 - HALO - 5                  # global t of audio col 0
    s0, s1 = max(0, g0), min(T, g0 + AW)
    audio[:, s0 - g0:s1 - g0] = inp["audio"][b, :, s0:s1]

    sel = np.zeros((76, SELW), np.float32)
    t = (t0 - HALO) + np.arange(SELW)
    valid = (t >= 0) & (t < T)
    tv = t[valid]
    sel[(tv + 1) // UP, np.where(valid)[0]] = inp["up_w"][(tv + 1) % UP]
    sel[75, valid] = 1.0

    m = {
        "audio_in": _bf(audio),
        "sel_in": _bf(sel),
        "mask_in": _bf(sel[75:76, :]),
        "aux_in": np.ascontiguousarray(inp["aux"][b]).astype(np.float32),
    }
    for k, v in w.items():
        m[k + "_in"] = np.ascontiguousarray(v)
    return m


def build_kernel():
    nc = bacc.Bacc(None, target_bir_lowering=False)
    d = {}
    shapes = {
        "audio_in": ((NQ, AW), BF), "sel_in": ((76, SELW), BF),
        "mask_in": ((1, SELW), BF),
        "aux_in": ((54, TAUX), F32),
        "wc0_in": ((128, 12, 128), BF), "wc1_in": ((128, 14, 128), BF),
        "bc0_in": ((1, 128), BF), "bc1a_in": ((1, 128), BF),
        "bc1b_in": ((1, 128), BF),
        "wd01_in": ((128, 9, 6, 384), BF), "wd2p_in": ((128, 9, 3, 384), BF),
        "bd_in": ((128, 9, 3), F32),
        "ws01_in": ((128, 9, 256), BF), "ws2_in": ((64, 9, 256), BF),
        "bss_in": ((128, 2), F32),
        "wsc_in": ((54, 54), F32), "bsc_in": ((54, 1), F32),
        "wa0_in": ((54, 3, 162), F32), "ba0_in": ((128, 2), F32),
        "wa1a_in": ((128, 3, NCOND), BF), "wa1b_in": ((34, 3, NCOND), BF),
        "ba1_in": ((128, 4), F32),
        "wi_in": ((9, 4, 128, 384), BF), "cb_in": ((9, 1, 384), BF),
        "wo1_in": ((128, 2, 256), BF), "bo1_in": ((128, 2), F32),
        "wo2_in": ((128, 2, 256), BF), "bo2row_in": ((1, 256), BF),
    }
    for k, (shp, dt) in shapes.items():
        d[k] = nc.dram_tensor(k, list(shp), dt, kind="ExternalInput")
    y_d = nc.dram_tensor("y", [TSH, NQ], F32, kind="ExternalOutput")

    def mm(out, lhsT, rhs, start, stop):
        nc.tensor.matmul(out, lhsT, rhs, start=start, stop=stop)

    with tile.TileContext(nc) as tc:
        with tc.tile_pool(name="res", bufs=1) as res:
            # ---- resident tiles + small DMAs ----
            sel_sb = res.tile([76, SELW], BF)
            nc.sync.dma_start(out=sel_sb, in_=d["sel_in"][:, :])
            mask_t = res.tile([1, SELW], BF)
            nc.sync.dma_start(out=mask_t, in_=d["mask_in"][:, :])
            mask = mask_t[:, :]
            bd_sb = res.tile([128, 9, 3], F32)
            nc.sync.dma_start(out=bd_sb, in_=d["bd_in"][:, :, :])
            ws01 = res.tile([128, 9, 256], BF)
            nc.sync.dma_start(out=ws01, in_=d["ws01_in"][:, :, :])
            ws2 = res.tile([64, 9, 256], BF)
            nc.sync.dma_start(out=ws2, in_=d["ws2_in"][:, :, :])
            bss_sb = res.tile([128, 2], F32)
            nc.sync.dma_start(out=bss_sb, in_=d["bss_in"][:, :])
            wo1 = res.tile([128, 2, 256], BF)
            nc.sync.dma_start(out=wo1, in_=d["wo1_in"][:, :, :])
            bo1_sb = res.tile([128, 2], F32)
            nc.sync.dma_start(out=bo1_sb, in_=d["bo1_in"][:, :])
            wo2 = res.tile([128, 2, 256], BF)
            nc.sync.dma_start(out=wo2, in_=d["wo2_in"][:, :, :])
            bo2row = res.tile([1, 256], BF)
            nc.sync.dma_start(out=bo2row, in_=d["bo2row_in"][:, :])

            # h ping-pong buffers (A = even-layer input, B = even-layer out)
            hA01 = res.tile([128, W], BF)
            hA2 = res.tile([128, W], BF)
            hB01 = res.tile([128, W], BF)
            hB2 = res.tile([128, W], BF)
            yaT = [res.tile([76, 384], BF, name=f"yaT{ll}") for ll in range(9)]
            wd01 = res.tile([128, 9, 6, 384], BF)
            wd2p = res.tile([128, 9, 3, 384], BF)

            # ------------- Phases A + B (scoped; audio freed after) --------
            with tc.tile_pool(name="pb", bufs=1) as pb, \
                 tc.tile_pool(name="pbs", bufs=2) as pbs, \
                 tc.tile_pool(name="ppB", bufs=1, space="PSUM") as ppB:
                # audio + causal weights first on the DMA queues
                a_t = [pb.tile([128, AW], BF, name=f"aud{i}") for i in range(2)]
                for i in range(2):
                    nc.sync.dma_start(
                        out=a_t[i][:, 0:1200],
                        in_=d["audio_in"][i * 128:(i + 1) * 128, 0:1200])
                wc0 = pb.tile([128, 12, 128], BF)
                nc.sync.dma_start(out=wc0, in_=d["wc0_in"][:, :, :])
                wc1 = pb.tile([128, 14, 128], BF)
                nc.sync.dma_start(out=wc1, in_=d["wc1_in"][:, :, :])
                bc0 = pb.tile([1, 128], BF)
                nc.sync.dma_start(out=bc0, in_=d["bc0_in"][:, :])
                bc1a = pb.tile([1, 128], BF)
                nc.sync.dma_start(out=bc1a, in_=d["bc1a_in"][:, :])
                bc1b = pb.tile([1, 128], BF)
                nc.sync.dma_start(out=bc1b, in_=d["bc1b_in"][:, :])
                for i in range(2):
                    nc.sync.dma_start(
                        out=a_t[i][:, 1200:AW],
                        in_=d["audio_in"][i * 128:(i + 1) * 128, 1200:AW])
                # big resident weight DMAs (queued behind audio)
                nc.sync.dma_start(out=wd01, in_=d["wd01_in"][:, :, :, :])
                nc.sync.dma_start(out=wd2p, in_=d["wd2p_in"][:, :, :, :])

                # ---------------- Phase A: conditioning ----------------
                with tc.tile_pool(name="ca", bufs=1) as ca, \
                     tc.tile_pool(name="cw", bufs=1) as cwp, \
                     tc.tile_pool(name="ppA", bufs=1, space="PSUM") as ppA:
                    aux_sb = ca.tile([54, TAUX], F32)
                    nc.sync.dma_start(out=aux_sb, in_=d["aux_in"][:, :])
                    wsc = ca.tile([54, 54], F32)
                    nc.sync.dma_start(out=wsc, in_=d["wsc_in"][:, :])
                    bsc = ca.tile([54, 1], F32)
                    nc.sync.dma_start(out=bsc, in_=d["bsc_in"][:, :])
                    ba0 = ca.tile([128, 2], F32)
                    nc.sync.dma_start(out=ba0, in_=d["ba0_in"][:, :])
                    ba1 = ca.tile([128, 4], F32)
                    nc.sync.dma_start(out=ba1, in_=d["ba1_in"][:, :])
                    wa0 = ca.tile([54, 3, 162], F32)
                    nc.sync.dma_start(out=wa0, in_=d["wa0_in"][:, :, :])
                    wa1a = ca.tile([128, 3, NCOND], BF)
                    nc.sync.dma_start(out=wa1a, in_=d["wa1a_in"][:, :, :])
                    wa1b = ca.tile([34, 3, NCOND], BF)
                    nc.sync.dma_start(out=wa1b, in_=d["wa1b_in"][:, :, :])

                    a0p = ppA.tile([54, TAUX], F32, tag="ap", bufs=2,
                                   padded_shape=P512)
                    mm(a0p, wsc, aux_sb, True, True)
                    a0 = ca.tile([54, TAUX], F32)
                    nc.scalar.activation(out=a0, in_=a0p, func=AF.Identity,
                                         bias=bsc)

                    a1blk = [(0, 128), (128, 34)]
                    a1 = [ca.tile([wd, TAUX], BF, name=f"a1_{i}")
                          for i, (o0, wd) in enumerate(a1blk)]
                    for i, (o0, wd) in enumerate(a1blk):
                        a1p = ppA.tile([wd, TAUX], F32, tag="ap", bufs=2,
                                       padded_shape=P512, name=f"a1p{i}")
                        ls = wa0[:, :, o0:o0 + wd]
                        mm(a1p, ls[:, 1, :], a0, True, False)
                        mm(a1p[:, 1:TAUX], ls[:, 0, :], a0[:, 0:TAUX - 1],
                           False, False)
                        mm(a1p[:, 0:TAUX - 1], ls[:, 2, :], a0[:, 1:TAUX],
                           False, True)
                        nc.scalar.activation(out=a1[i], in_=a1p,
                                             func=AF.Identity,
                                             bias=ba0[0:wd, i:i + 1])

                    a2blk = [(0, 128), (128, 128), (256, 128), (384, 102)]
                    a2 = [ca.tile([wd, TAUX], BF, name=f"a2_{i}")
                          for i, (o0, wd) in enumerate(a2blk)]
                    for i, (o0, wd) in enumerate(a2blk):
                        a2p = ppA.tile([wd, TAUX], F32, tag="ap", bufs=2,
                                       padded_shape=P512, name=f"a2p{i}")
                        for kc, wsrc in enumerate([wa1a, wa1b]):
                            ls = wsrc[:, :, o0:o0 + wd]
                            rhs = a1[kc]
                            mm(a2p, ls[:, 1, :], rhs, kc == 0, False)
                            mm(a2p[:, 3:TAUX], ls[:, 0, :], rhs[:, 0:TAUX - 3],
                               False, False)
                            mm(a2p[:, 0:TAUX - 3], ls[:, 2, :], rhs[:, 3:TAUX],
                               False, kc == 1)
                        nc.scalar.activation(out=a2[i], in_=a2p,
                                             func=AF.Identity,
                                             bias=ba1[0:wd, i:i + 1])

                    for ll in range(9):
                        wi_sb = cwp.tile([128, 4, 384], BF, tag="wi")
                        nc.sync.dma_start(
                            out=wi_sb,
                            in_=d["wi_in"][ll, :, :, :].rearrange(
                                "r p n -> p r n"))
                        yp = ppA.tile([TAUX, 384], F32, tag="yp", bufs=2,
                                      padded_shape=P512, name=f"yp{ll}")
                        for r, (o0, wd) in enumerate(a2blk):
                            mm(yp, a2[r], wi_sb[0:wd, r, :], r == 0, r == 3)
                        nc.scalar.activation(out=yaT[ll][0:TAUX, :], in_=yp,
                                             func=AF.Copy)
                        nc.sync.dma_start(out=yaT[ll][TAUX:76, :],
                                          in_=d["cb_in"][ll, :, :])

                # ------------- Phase B: causal conv + softsign -------------
                for (c0, w_) in _subtiles(0, W):
                    cc0 = ppB.tile([128, w_], F32, tag="cc", bufs=3,
                                   padded_shape=P512, name=f"cc0_{c0}")
                    for tap in range(6):
                        for rc in range(2):
                            mm(cc0, wc0[:, tap * 2 + rc, :],
                               a_t[rc][:, c0 + tap:c0 + tap + w_],
                               tap == 0 and rc == 0, False)
                    mm(cc0, bc0, mask[:, c0:c0 + w_], False, True)
                    cc1 = ppB.tile([128, w_], F32, tag="cc", bufs=3,
                                   padded_shape=P512, name=f"cc1_{c0}")
                    for tau in range(7):
                        for rc in range(2):
                            mm(cc1, wc1[:, tau * 2 + rc, :],
                               a_t[rc][:, c0 + tau:c0 + tau + w_],
                               tau == 0 and rc == 0, False)
                    mm(cc1, bc1a, mask[:, c0:c0 + w_], False, False)
                    mm(cc1, bc1b, mask[:, c0 + 1:c0 + 1 + w_], False, True)
                    for ci, (ccp, dst) in enumerate(((cc0, hA01), (cc1, hA2))):
                        ab = pbs.tile([128, w_], F32, tag="ab",
                                      padded_shape=P512)
                        nc.scalar.activation(out=ab, in_=ccp, func=AF.Abs)
                        nc.vector.tensor_scalar(out=ab, in0=ab, scalar1=1.0,
                                                scalar2=None, op0=ALU.add)
                        rr = pbs.tile([128, w_], F32, tag="rr",
                                      padded_shape=P512)
                        nc.vector.reciprocal_approx_fast(out=rr, in_=ab)
                        nc.vector.tensor_tensor(
                            out=dst[:, c0:c0 + w_], in0=ccp, in1=rr,
                            op=ALU.mult)

            # ---------------- Phases C + D ----------------
            with tc.tile_pool(name="scr", bufs=2) as scr, \
                 tc.tile_pool(name="od", bufs=1) as od, \
                 tc.tile_pool(name="pc", bufs=1, space="PSUM") as pc:
                ss = [od.tile([128, TSH], F32, name=f"ss{i}")
                      for i in range(2)]
                # layer-7 h buffers are dead once layer 8 has read them;
                # reuse as r1 storage (D trails layer 8 by >1 subtile)
                r1 = [hA01[:, 0:TSH], hA2[:, 0:TSH]]
                pend = [None]
                d_subs = _subtiles(OUT0, W)
                d_idx = [0]
                q0r = [OUT0]

                def emit_skip(ll, c0, w_, c01, c2):
                    s0 = max(c0, OUT0)
                    if s0 >= c0 + w_:
                        return
                    wv = c0 + w_ - s0
                    for ob in range(2):
                        skp = pc.tile([128, wv], F32, tag="sk", bufs=2,
                                      padded_shape=P512,
                                      name=f"skp{ll}_{c0}_{ob}")
                        mm(skp, ws01[:, ll, ob * 128:(ob + 1) * 128],
                           c01[:, s0:s0 + wv], True, False)
                        mm(skp, ws2[:, ll, ob * 128:(ob + 1) * 128],
                           c2[0:64, s0:s0 + wv], False, True)
                        sv = ss[ob][:, s0 - OUT0:s0 - OUT0 + wv]
                        if ll == 0:
                            nc.scalar.activation(out=sv, in_=skp,
                                                 func=AF.Identity,
                                                 bias=bss_sb[:, ob:ob + 1])
                        else:
                            nc.vector.tensor_tensor(out=sv, in0=skp, in1=sv,
                                                    op=ALU.add)

                def emit_o2(q0, qw):
                    o2p = pc.tile([qw, 256], F32, tag="sk", bufs=2,
                                  padded_shape=P512, name=f"o2p{q0}")
                    mm(o2p, r1[0][:, q0 - OUT0:q0 - OUT0 + qw],
                       wo2[:, 0, :], True, False)
                    mm(o2p, r1[1][:, q0 - OUT0:q0 - OUT0 + qw],
                       wo2[:, 1, :], False, False)
                    mm(o2p, mask[:, q0:q0 + qw], bo2row, False, True)
                    og = od.tile([qw, 256], F32, tag="og", bufs=2,
                                 padded_shape=[128, 256])
                    nc.scalar.activation(out=og, in_=o2p, func=AF.Copy)
                    nc.sync.dma_start(
                        out=y_d[q0 - OUT0:q0 - OUT0 + qw, :], in_=og)

                def emit_d_sub(c0, w_):
                    rl = []
                    for kc in range(2):
                        rt = scr.tile([128, w_], BF, tag=f"rl{kc}",
                                      padded_shape=P512)
                        nc.scalar.activation(
                            out=rt,
                            in_=ss[kc][:, c0 - OUT0:c0 - OUT0 + w_],
                            func=AF.Relu)
                        rl.append(rt)
                    for ob in range(2):
                        o1p = pc.tile([128, w_], F32, tag="xc", bufs=3,
                                      padded_shape=P512, name=f"o1p{c0}_{ob}")
                        mm(o1p, wo1[:, 0, ob * 128:(ob + 1) * 128],
                           rl[0], True, False)
                        mm(o1p, wo1[:, 1, ob * 128:(ob + 1) * 128],
                           rl[1], False, True)
                        nc.scalar.activation(
                            out=r1[ob][:, c0 - OUT0:c0 - OUT0 + w_],
                            in_=o1p, func=AF.Relu,
                            bias=bo1_sb[:, ob:ob + 1])

                def advance_d(x):
                    # emit D subtiles fully covered by skip-complete cols < x
                    while (d_idx[0] < len(d_subs)
                           and d_subs[d_idx[0]][0] + d_subs[d_idx[0]][1] <= x):
                        c0d, wd = d_subs[d_idx[0]]
                        emit_d_sub(c0d, wd)
                        d_idx[0] += 1
                        while q0r[0] + 128 <= c0d:
                            emit_o2(q0r[0], 128)
                            q0r[0] += 128

                def flush_pend():
                    if pend[0] is not None:
                        emit_skip(*pend[0])
                        if pend[0][0] == 8:
                            advance_d(pend[0][1] + pend[0][2])
                        pend[0] = None

                for ll in range(9):
                    dl = DILS[ll]
                    dnx = DILS[ll + 1] if ll < 8 else 0
                    if ll % 2 == 0:
                        prev01, prev2, cur01, cur2 = hA01, hA2, hB01, hB2
                    else:
                        prev01, prev2, cur01, cur2 = hB01, hB2, hA01, hA2
                    subs = _subtiles(OL[ll], W)
                    pairs = [subs[i:i + 2] for i in range(0, len(subs), 2)]
                    for pair in pairs:
                        xcsm = {}
                        for (c0, w_) in pair:
                            xcs = []
                            for mb in range(3):
                                xcp = pc.tile([128, w_], F32, tag="xc",
                                              bufs=3, padded_shape=P512,
                                              name=f"xcp{ll}_{c0}_{mb}")
                                mm(xcp, yaT[ll][:, mb * 128:(mb + 1) * 128],
                                   sel_sb[:, c0:c0 + w_], True, True)
                                xc_sb = scr.tile([128, w_], BF,
                                                 tag=f"xcs{mb}",
                                                 padded_shape=P512)
                                nc.scalar.activation(out=xc_sb, in_=xcp,
                                                     func=AF.Copy)
                                xcs.append(xc_sb)
                            xcsm[c0] = xcs
                        xhm = {c0: [] for (c0, w_) in pair}
                        for mb in range(3):
                            hcps = {}
                            for (c0, w_) in pair:
                                hcps[c0] = pc.tile(
                                    [128, w_], F32, tag="hc", bufs=3,
                                    padded_shape=P512,
                                    name=f"hcp{ll}_{c0}_{mb}")
                            for k in range(9):
                                for (c0, w_) in pair:
                                    if k < 6:
                                        off = (k - 5) * dl
                                        mm(hcps[c0],
                                           wd01[:, ll, k,
                                                mb * 128:(mb + 1) * 128],
                                           prev01[:, c0 + off:c0 + off + w_],
                                           k == 0, False)
                                    else:
                                        j = k - 6
                                        off = (2 * j - 5) * dl
                                        mm(hcps[c0],
                                           wd2p[:, ll, j,
                                                mb * 128:(mb + 1) * 128],
                                           prev2[:, c0 + off:c0 + off + w_],
                                           False, j == 2)
                            for (c0, w_) in pair:
                                xh_sb = scr.tile([128, w_], BF,
                                                 tag=f"xh{mb}",
                                                 padded_shape=P512)
                                nc.vector.scalar_tensor_tensor(
                                    out=xh_sb, in0=hcps[c0],
                                    scalar=bd_sb[:, ll, mb:mb + 1],
                                    in1=xcsm[c0][mb],
                                    op0=ALU.add, op1=ALU.mult)
                                xhm[c0].append(xh_sb)
                        for (c0, w_) in pair:
                            xh = xhm[c0]
                            flush_pend()
                            xh2b = scr.tile([64, w_], BF, tag="xh2b",
                                            padded_shape=P512)
                            nc.sync.dma_start(out=xh2b, in_=xh[2][64:128, :])
                            nc.scalar.activation(out=xh[0], in_=xh[0],
                                                 func=AF.Sigmoid)
                            nc.scalar.activation(out=xh[1], in_=xh[1],
                                                 func=AF.Tanh)
                            nc.scalar.activation(out=xh[2][0:64, :],
                                                 in_=xh[2][0:64, :],
                                                 func=AF.Sigmoid)
                            nc.scalar.activation(out=xh2b, in_=xh2b,
                                                 func=AF.Tanh)
                            dd = scr.tile([128, w_], BF, tag="dd",
                                          padded_shape=P512)
                            nc.vector.tensor_tensor(
                                out=dd, in0=prev01[:, c0:c0 + w_],
                                in1=xh[1], op=ALU.subtract)
                            nc.vector.tensor_tensor(out=dd, in0=xh[0],
                                                    in1=dd, op=ALU.mult)
                            nc.vector.tensor_tensor(
                                out=cur01[:, c0:c0 + w_],
                                in0=xh[1], in1=dd, op=ALU.add)
                            dd2 = scr.tile([64, w_], BF, tag="dd2",
                                           padded_shape=P512)
                            nc.vector.tensor_tensor(
                                out=dd2, in0=prev2[0:64, c0:c0 + w_],
                                in1=xh2b, op=ALU.subtract)
                            nc.vector.tensor_tensor(out=dd2,
                                                    in0=xh[2][0:64, :],
                                                    in1=dd2, op=ALU.mult)
                            nc.vector.tensor_tensor(
                                out=cur2[0:64, c0:c0 + w_],
                                in0=xh2b, in1=dd2, op=ALU.add)
                            if dnx:
                                nc.sync.dma_start(
                                    out=cur2[64:128, c0 - dnx:c0 - dnx + w_],
                                    in_=cur2[0:64, c0:c0 + w_])
                            pend[0] = (ll, c0, w_, cur01, cur2)
                flush_pend()
                advance_d(W + 1)
                while q0r[0] < W:
                    qw = min(128, W - q0r[0])
                    emit_o2(q0r[0], qw)
                    q0r[0] += qw
    nc.compile()
    return nc


_NC_CACHE = {}


def kernel(**inputs):
    inp = {k: np.ascontiguousarray(np.asarray(v, dtype=np.float32))
           for k, v in inputs.items()}
    if "nc" not in _NC_CACHE:
        _NC_CACHE["nc"] = build_kernel()
    nc = _NC_CACHE["nc"]
    w = _pack_weights(inp)
    in_maps = [_per_core_arrays(inp, w, core // 2, core % 2)
               for core in range(8)]
    res = run_bass_kernel_spmd(nc, in_maps, core_ids=list(range(8)))
    out = np.empty((B, T, NQ), np.float32)
    for core in range(8):
        b, half = core // 2, core % 2
        y = res.results[core]["y"]
        if half == 0:
            out[b, 0:TSH] = y
        else:
            out[b, TSH:T] = y[0:T - TSH]
    return out
